# revision 22
# baseline (speedup 1.0000x reference)
"""Trainium2 Bass kernel for nn_BulkSpaceGenerator.

Math: the fast-marching scan g_k = g_{k-1} + (1/(k+1))(c_k - g_{k-1}) starting
from c_0 yields the running mean g_k = mean(c_0..c_k); the mean over k of those
is sum_j w_j c_j with w_j = (1/K)(H_K - H_j) (harmonic numbers). Since
c_j = tokens @ W[:, j*D:(j+1)*D] + b[j*D:(j+1)*D], the whole module is

    out = tokens @ W_eff + b_eff,   W_eff = sum_j w_j W_j,  b_eff = sum_j w_j b_j

W_eff/b_eff are constant-folded from the weights on the host during input
formatting (the same pass that casts to f16 and transposes); the device then
runs the (8192x1024)@(1024x1024) matmul on the PE array at the f16 roofline,
sharded over 8 cores as 4 feature-shards x 2 token-shards.

Schedule (PE-bound, ~216ns per 128x512 matmul):
  - sync ring: W_eff slice (one 0.5MB transfer), then 16 token half-tiles.
  - two psum generations of [4 m-chunks x 2 d-tiles = 8 banks], kt-outer;
    generation 2 reuses banks as generation 1's groups evict.
  - evictions (psum + bias -> f16): dt0 on ACT -> scalar-ring DMA, dt1 on DVE
    -> gpsimd SWDGE DMA, so out bytes never queue behind token loads.

Layout per core (f in 0..3, t in 0..1, core = f*2 + t):
  tokT : (1024, 4096) f16 -- tokens^T slice, columns t*4096:(t+1)*4096
  weff : (128, 2048)  f16 -- W_eff[kt*128+p, f*256+d] at [p, kt*256+d]
  beff : (256, 1)     f32 -- b_eff slice
  outT : (256, 4096)  f16 -- out^T slice (host reassembles (4,2048,1024))
"""

import os
from contextlib import ExitStack

import numpy as np

import concourse.bass as bass
import concourse.tile as tile
from concourse import bacc, mybir
from concourse.bass_utils import run_bass_kernel_spmd

D_MODEL = 1024
BULK_DIM = 10
B, N = 4, 2048
BN = B * N                     # 8192 tokens
NCORES = 8
F_SHARDS = 4                   # feature shards (d dimension)
T_SHARDS = 2                   # token shards
DS = D_MODEL // F_SHARDS       # 256 output features per core
MS = BN // T_SHARDS            # 4096 tokens per core
KT = D_MODEL // 128            # 8 contraction k-tiles
DT = DS // 128                 # 2 output d-tiles of 128 per core
MCHUNK = 512                   # moving free dim per matmul
NMI = MS // MCHUNK             # 8 m-chunks per core
HM = MS // 2                   # 2048 tokens per generation
GMI = NMI // 2                 # 4 m-chunks per generation

# w_j = (1/K) * (H_K - H_j), H_j = sum_{i=1..j} 1/i
_H = np.cumsum(1.0 / np.arange(1, BULK_DIM + 1))
W_COEF = ((_H[-1] - np.concatenate([[0.0], _H[:-1]])) / BULK_DIM).tolist()

MODE = os.environ.get("BULK_KERNEL_MODE", "host")

_BUILD_CACHE = {}

N_PREWARM = 18                 # PE warm-up no-op matmuls before the stream


def _build(mode: str) -> bass.Bass:
    f32 = mybir.dt.float32
    f16 = mybir.dt.float16

    nc = bacc.Bacc("TRN2", target_bir_lowering=False, debug=False,
                   num_devices=NCORES)
    tokT = nc.dram_tensor("tokT", [D_MODEL, MS], f16,
                          kind="ExternalInput").ap()
    weff = nc.dram_tensor("weff", [128, KT * DS], f16,
                          kind="ExternalInput").ap()
    beff = nc.dram_tensor("beff", [DS, 1], f32, kind="ExternalInput").ap()
    outT = nc.dram_tensor("outT", [DS, MS], f16, kind="ExternalOutput").ap()

    with tile.TileContext(nc) as tc, ExitStack() as ctx:
        weff_pool = ctx.enter_context(tc.tile_pool(name="weff", bufs=1))
        tok_pool = ctx.enter_context(tc.tile_pool(name="tok", bufs=2 * KT))
        bias_pool = ctx.enter_context(tc.tile_pool(name="bias", bufs=DT))
        zero_pool = ctx.enter_context(tc.tile_pool(name="zero", bufs=2))
        psum_pool = ctx.enter_context(
            tc.tile_pool(name="psum", bufs=8, space="PSUM"))
        out_pool = ctx.enter_context(tc.tile_pool(name="osb", bufs=8))

        # ---- weights lead the gpsimd SWDGE ring; bias leads scalar ----
        wt = weff_pool.tile([128, KT * DS], f16, tag="wt")
        nc.gpsimd.dma_start(wt[:], weff[:, :])

        biases = []
        for dt_i in range(DT):
            bt = bias_pool.tile([128, 1], f32, tag="bt")
            nc.scalar.dma_start(bt[:], beff[dt_i * 128:(dt_i + 1) * 128, :])
            biases.append(bt)

        # token half-tiles round-robin across all THREE rings (sync/scalar
        # HWDGE + gpsimd SWDGE): the stream ramp is per-ring cadence-bound
        # (~2.5us per 512KB transfer incl completion), so three rings cut the
        # early feed latency by ~a third
        rings = [nc.sync, nc.scalar, nc.gpsimd]
        toks = [[None] * KT for _ in range(2)]   # [half][kt]
        for h in range(2):
            for kt in range(KT):
                tk = tok_pool.tile([128, HM], f16, tag="tk")
                rings[(h * KT + kt) % 3].dma_start(
                    tk[:], tokT[kt * 128:(kt + 1) * 128,
                                h * HM:(h + 1) * HM])
                toks[h][kt] = tk

        # ---- zero operands for PE-warming no-op matmuls ----
        zmm = zero_pool.tile([128, 128], f16, tag="zmm")
        nc.gpsimd.memset(zmm[:], 0.0)
        zrhs = zero_pool.tile([128, MCHUNK], f16, tag="zrhs")
        nc.gpsimd.memset(zrhs[:], 0.0)

        def evict_pair(ps_a, ps_b, dt_i, moff):
            # two adjacent m-chunks of one d-tile -> one [128,1024] out tile
            # -> one 262KB DMA; dt0 rides the scalar HWDGE ring, dt1 the sync
            # HWDGE ring (both idle by the time the bulk of the outs flow)
            ot = out_pool.tile([128, 2 * MCHUNK], f16, name="ot", tag="ot")
            if dt_i == 0:
                nc.scalar.add(ot[:, 0:MCHUNK], ps_a[:], biases[dt_i][:])
                nc.scalar.add(ot[:, MCHUNK:], ps_b[:], biases[dt_i][:])
                nc.scalar.dma_start(
                    outT[dt_i * 128:(dt_i + 1) * 128,
                         moff:moff + 2 * MCHUNK], ot[:])
            else:
                nc.vector.tensor_scalar_add(
                    ot[:, 0:MCHUNK], ps_a[:], biases[dt_i][:, 0:1])
                nc.vector.tensor_scalar_add(
                    ot[:, MCHUNK:], ps_b[:], biases[dt_i][:, 0:1])
                nc.sync.dma_start(
                    outT[dt_i * 128:(dt_i + 1) * 128,
                         moff:moff + 2 * MCHUNK], ot[:])

        # ---- generation 1: [4 m-chunks x 2 d-tiles = 8 banks] on token half
        # A, kt-outer (8 matmuls per k-step tracks the token arrival rate);
        # then two 4-bank sub-generations on half B, so the first sub-gen's
        # evictions overlap the second's matmuls and only two eviction pairs
        # trail the stream ----
        psums = [[psum_pool.tile([128, MCHUNK], f32, name="ps", tag="ps")
                  for _ in range(DT)] for _ in range(GMI)]
        for _ in range(N_PREWARM):
            nc.tensor.matmul(psums[0][0][:], lhsT=zmm[:], rhs=zrhs[:],
                             start=False, stop=False)
        for kt in range(KT):
            for dt_i in range(DT):
                lhsT = wt[:, kt * DS + dt_i * 128:
                          kt * DS + (dt_i + 1) * 128]
                for mi in range(GMI):
                    nc.tensor.matmul(
                        psums[mi][dt_i][:],
                        lhsT=lhsT,
                        rhs=toks[0][kt][:, mi * MCHUNK:(mi + 1) * MCHUNK],
                        start=(kt == 0), stop=(kt == KT - 1))
        for mi in range(0, GMI, 2):
            for dt_i in range(DT):
                evict_pair(psums[mi][dt_i], psums[mi + 1][dt_i], dt_i,
                           mi * MCHUNK)

        for q in range(2):
            psums = [[psum_pool.tile([128, MCHUNK], f32, name="ps", tag="ps")
                      for _ in range(DT)] for _ in range(2)]
            for kt in range(KT):
                for dt_i in range(DT):
                    lhsT = wt[:, kt * DS + dt_i * 128:
                              kt * DS + (dt_i + 1) * 128]
                    for mi in range(2):
                        moff = (q * 2 + mi) * MCHUNK
                        nc.tensor.matmul(
                            psums[mi][dt_i][:],
                            lhsT=lhsT,
                            rhs=toks[1][kt][:, moff:moff + MCHUNK],
                            start=(kt == 0), stop=(kt == KT - 1))
            for dt_i in range(DT):
                evict_pair(psums[0][dt_i], psums[1][dt_i], dt_i,
                           HM + q * 2 * MCHUNK)

    nc.compile()
    return nc


def _get_nc(mode: str) -> bass.Bass:
    if mode not in _BUILD_CACHE:
        _BUILD_CACHE[mode] = _build(mode)
    return _BUILD_CACHE[mode]


def _make_in_maps(boundary_tokens, W_b2b, b_b2b):
    wcoef = np.asarray(W_COEF, dtype=np.float32)
    tok = np.ascontiguousarray(
        np.asarray(boundary_tokens, dtype=np.float32)
        .reshape(BN, D_MODEL).T.astype(np.float16))
    # constant-fold the scan into the weights: W_eff = sum_j w_j W_j
    Weff = (np.asarray(W_b2b, dtype=np.float32).reshape(
        D_MODEL, BULK_DIM, D_MODEL) * wcoef[None, :, None]).sum(
        axis=1, dtype=np.float32)
    beff = (np.asarray(b_b2b, dtype=np.float32).reshape(BULK_DIM, D_MODEL)
            * wcoef[:, None]).sum(axis=0, dtype=np.float32)
    Weff16 = Weff.astype(np.float16).reshape(KT, 128, D_MODEL)
    in_maps = []
    for c in range(NCORES):
        f, t = divmod(c, T_SHARDS)
        dsl = slice(f * DS, (f + 1) * DS)
        in_maps.append({
            "tokT": np.ascontiguousarray(tok[:, t * MS:(t + 1) * MS]),
            "weff": np.ascontiguousarray(
                Weff16[:, :, dsl].transpose(1, 0, 2).reshape(128, KT * DS)),
            "beff": np.ascontiguousarray(beff[dsl, None]),
        })
    return in_maps


def _assemble(results):
    out = np.empty((BN, D_MODEL), dtype=np.float32)
    for c in range(NCORES):
        f, t = divmod(c, T_SHARDS)
        out[t * MS:(t + 1) * MS, f * DS:(f + 1) * DS] = \
            results[c]["outT"].T.astype(np.float32)
    return out.reshape(B, N, D_MODEL)


def run(boundary_tokens, W_b2b, b_b2b, mode=None, **spmd_kwargs):
    mode = mode or MODE
    nc = _get_nc(mode)
    in_maps = _make_in_maps(boundary_tokens, W_b2b, b_b2b)
    res = run_bass_kernel_spmd(nc, in_maps, list(range(NCORES)), **spmd_kwargs)
    return _assemble(res.results), res


def kernel(boundary_tokens, W_b2b, b_b2b):
    out, _ = run(boundary_tokens, W_b2b, b_b2b)
    return out


# revision 25
# speedup vs baseline: 1.1351x; 1.1351x over previous
"""Trainium2 Bass kernel for nn_BulkSpaceGenerator.

Math: the fast-marching scan g_k = g_{k-1} + (1/(k+1))(c_k - g_{k-1}) starting
from c_0 yields the running mean g_k = mean(c_0..c_k); the mean over k of those
is sum_j w_j c_j with w_j = (1/K)(H_K - H_j) (harmonic numbers). Since
c_j = tokens @ W[:, j*D:(j+1)*D] + b[j*D:(j+1)*D], the whole module is

    out = tokens @ W_eff + b_eff,   W_eff = sum_j w_j W_j,  b_eff = sum_j w_j b_j

W_eff/b_eff are constant-folded from the weights on the host during input
formatting (the same pass that casts to f16 and transposes); the device then
runs the (8192x1024)@(1024x1024) matmul on the PE array at the f16 roofline,
sharded over 8 cores as 4 feature-shards x 2 token-shards.

Schedule (PE-bound, ~216ns per 128x512 matmul):
  - sync ring: W_eff slice (one 0.5MB transfer), then 16 token half-tiles.
  - two psum generations of [4 m-chunks x 2 d-tiles = 8 banks], kt-outer;
    generation 2 reuses banks as generation 1's groups evict.
  - evictions (psum + bias -> f16): dt0 on ACT -> scalar-ring DMA, dt1 on DVE
    -> gpsimd SWDGE DMA, so out bytes never queue behind token loads.

Layout per core (f in 0..3, t in 0..1, core = f*2 + t):
  tokT : (1024, 4096) f16 -- tokens^T slice, columns t*4096:(t+1)*4096
  weff : (128, 2048)  f16 -- W_eff[kt*128+p, f*256+d] at [p, kt*256+d]
  beff : (256, 1)     f32 -- b_eff slice
  outT : (256, 4096)  f16 -- out^T slice (host reassembles (4,2048,1024))
"""

import os
from contextlib import ExitStack

import numpy as np

import concourse.bass as bass
import concourse.tile as tile
from concourse import bacc, mybir
from concourse.bass_utils import run_bass_kernel_spmd

D_MODEL = 1024
BULK_DIM = 10
B, N = 4, 2048
BN = B * N                     # 8192 tokens
NCORES = 8
F_SHARDS = 4                   # feature shards (d dimension)
T_SHARDS = 2                   # token shards
DS = D_MODEL // F_SHARDS       # 256 output features per core
MS = BN // T_SHARDS            # 4096 tokens per core
KT = D_MODEL // 128            # 8 contraction k-tiles
DT = DS // 128                 # 2 output d-tiles of 128 per core
MCHUNK = 512                   # moving free dim per matmul
NMI = MS // MCHUNK             # 8 m-chunks per core
HM = MS // 2                   # 2048 tokens per generation
GMI = NMI // 2                 # 4 m-chunks per generation

# w_j = (1/K) * (H_K - H_j), H_j = sum_{i=1..j} 1/i
_H = np.cumsum(1.0 / np.arange(1, BULK_DIM + 1))
W_COEF = ((_H[-1] - np.concatenate([[0.0], _H[:-1]])) / BULK_DIM).tolist()

MODE = os.environ.get("BULK_KERNEL_MODE", "host")

_BUILD_CACHE = {}

N_PREWARM = 10                 # PE warm-up no-op matmuls before the stream


def _build(mode: str) -> bass.Bass:
    f32 = mybir.dt.float32
    f16 = mybir.dt.float16

    nc = bacc.Bacc("TRN2", target_bir_lowering=False, debug=False,
                   num_devices=NCORES)
    tokT = nc.dram_tensor("tokT", [D_MODEL, MS], f16,
                          kind="ExternalInput").ap()
    weff = nc.dram_tensor("weff", [128, KT * DS], f16,
                          kind="ExternalInput").ap()
    beff = nc.dram_tensor("beff", [DS, 1], f32, kind="ExternalInput").ap()
    outT = nc.dram_tensor("outT", [DS, MS], f16, kind="ExternalOutput").ap()

    with tile.TileContext(nc) as tc, ExitStack() as ctx:
        weff_pool = ctx.enter_context(tc.tile_pool(name="weff", bufs=1))
        tok_pool = ctx.enter_context(tc.tile_pool(name="tok", bufs=2 * KT))
        bias_pool = ctx.enter_context(tc.tile_pool(name="bias", bufs=DT))
        zero_pool = ctx.enter_context(tc.tile_pool(name="zero", bufs=2))
        psum_pool = ctx.enter_context(
            tc.tile_pool(name="psum", bufs=8, space="PSUM"))
        out_pool = ctx.enter_context(tc.tile_pool(name="osb", bufs=8))

        # ---- two HWDGE rings, interleaved so the k-order arrival tracks the
        # PE's consumption: sync = [weff, A1, A3, A5, A7, B...], scalar =
        # [A0, A2, A4, A6, bias, B...]; bias rides late (needed only at the
        # first eviction) so it never delays a token transfer ----
        wt = weff_pool.tile([128, KT * DS], f16, tag="wt")
        nc.sync.dma_start(wt[:], weff[:, :])

        toks = [[None] * KT for _ in range(2)]   # [half][kt]
        for kt in range(KT):
            tk = tok_pool.tile([128, HM], f16, tag="tk")
            eng = nc.scalar if kt % 2 == 0 else nc.sync
            eng.dma_start(tk[:], tokT[kt * 128:(kt + 1) * 128, 0:HM])
            toks[0][kt] = tk

        biases = []
        for dt_i in range(DT):
            bt = bias_pool.tile([128, 1], f32, tag="bt")
            nc.scalar.dma_start(bt[:], beff[dt_i * 128:(dt_i + 1) * 128, :])
            biases.append(bt)

        for kt in range(KT):
            tk = tok_pool.tile([128, HM], f16, tag="tk")
            eng = nc.scalar if kt % 2 == 0 else nc.sync
            eng.dma_start(tk[:], tokT[kt * 128:(kt + 1) * 128, HM:MS])
            toks[1][kt] = tk

        # ---- zero operands for PE-warming no-op matmuls ----
        zmm = zero_pool.tile([128, 128], f16, tag="zmm")
        nc.gpsimd.memset(zmm[:], 0.0)
        zrhs = zero_pool.tile([128, MCHUNK], f16, tag="zrhs")
        nc.gpsimd.memset(zrhs[:], 0.0)

        def evict_pair(ps_a, ps_b, dt_i, moff):
            # two adjacent m-chunks of one d-tile -> one [128,1024] out tile
            # -> one 262KB DMA; dt0 rides the scalar HWDGE ring, dt1 the sync
            # HWDGE ring (both idle by the time the bulk of the outs flow)
            ot = out_pool.tile([128, 2 * MCHUNK], f16, name="ot", tag="ot")
            if dt_i == 0:
                nc.scalar.add(ot[:, 0:MCHUNK], ps_a[:], biases[dt_i][:])
                nc.scalar.add(ot[:, MCHUNK:], ps_b[:], biases[dt_i][:])
                nc.scalar.dma_start(
                    outT[dt_i * 128:(dt_i + 1) * 128,
                         moff:moff + 2 * MCHUNK], ot[:])
            else:
                nc.vector.tensor_scalar_add(
                    ot[:, 0:MCHUNK], ps_a[:], biases[dt_i][:, 0:1])
                nc.vector.tensor_scalar_add(
                    ot[:, MCHUNK:], ps_b[:], biases[dt_i][:, 0:1])
                nc.sync.dma_start(
                    outT[dt_i * 128:(dt_i + 1) * 128,
                         moff:moff + 2 * MCHUNK], ot[:])

        # ---- generation 1: [4 m-chunks x 2 d-tiles = 8 banks] on token half
        # A, kt-outer (8 matmuls per k-step tracks the token arrival rate);
        # then two 4-bank sub-generations on half B, so the first sub-gen's
        # evictions overlap the second's matmuls and only two eviction pairs
        # trail the stream ----
        psums = [[psum_pool.tile([128, MCHUNK], f32, name="ps", tag="ps")
                  for _ in range(DT)] for _ in range(GMI)]
        for _ in range(N_PREWARM):
            nc.tensor.matmul(psums[0][0][:], lhsT=zmm[:], rhs=zrhs[:],
                             start=False, stop=False)
        for kt in range(KT):
            for dt_i in range(DT):
                lhsT = wt[:, kt * DS + dt_i * 128:
                          kt * DS + (dt_i + 1) * 128]
                for mi in range(GMI):
                    nc.tensor.matmul(
                        psums[mi][dt_i][:],
                        lhsT=lhsT,
                        rhs=toks[0][kt][:, mi * MCHUNK:(mi + 1) * MCHUNK],
                        start=(kt == 0), stop=(kt == KT - 1))
        for mi in range(0, GMI, 2):
            for dt_i in range(DT):
                evict_pair(psums[mi][dt_i], psums[mi + 1][dt_i], dt_i,
                           mi * MCHUNK)

        for q in range(2):
            psums = [[psum_pool.tile([128, MCHUNK], f32, name="ps", tag="ps")
                      for _ in range(DT)] for _ in range(2)]
            for kt in range(KT):
                for dt_i in range(DT):
                    lhsT = wt[:, kt * DS + dt_i * 128:
                              kt * DS + (dt_i + 1) * 128]
                    for mi in range(2):
                        moff = (q * 2 + mi) * MCHUNK
                        nc.tensor.matmul(
                            psums[mi][dt_i][:],
                            lhsT=lhsT,
                            rhs=toks[1][kt][:, moff:moff + MCHUNK],
                            start=(kt == 0), stop=(kt == KT - 1))
            for dt_i in range(DT):
                evict_pair(psums[0][dt_i], psums[1][dt_i], dt_i,
                           HM + q * 2 * MCHUNK)

    nc.compile()
    return nc


def _get_nc(mode: str) -> bass.Bass:
    if mode not in _BUILD_CACHE:
        _BUILD_CACHE[mode] = _build(mode)
    return _BUILD_CACHE[mode]


def _make_in_maps(boundary_tokens, W_b2b, b_b2b):
    wcoef = np.asarray(W_COEF, dtype=np.float32)
    tok = np.ascontiguousarray(
        np.asarray(boundary_tokens, dtype=np.float32)
        .reshape(BN, D_MODEL).T.astype(np.float16))
    # constant-fold the scan into the weights: W_eff = sum_j w_j W_j
    Weff = (np.asarray(W_b2b, dtype=np.float32).reshape(
        D_MODEL, BULK_DIM, D_MODEL) * wcoef[None, :, None]).sum(
        axis=1, dtype=np.float32)
    beff = (np.asarray(b_b2b, dtype=np.float32).reshape(BULK_DIM, D_MODEL)
            * wcoef[:, None]).sum(axis=0, dtype=np.float32)
    Weff16 = Weff.astype(np.float16).reshape(KT, 128, D_MODEL)
    in_maps = []
    for c in range(NCORES):
        f, t = divmod(c, T_SHARDS)
        dsl = slice(f * DS, (f + 1) * DS)
        in_maps.append({
            "tokT": np.ascontiguousarray(tok[:, t * MS:(t + 1) * MS]),
            "weff": np.ascontiguousarray(
                Weff16[:, :, dsl].transpose(1, 0, 2).reshape(128, KT * DS)),
            "beff": np.ascontiguousarray(beff[dsl, None]),
        })
    return in_maps


def _assemble(results):
    out = np.empty((BN, D_MODEL), dtype=np.float32)
    for c in range(NCORES):
        f, t = divmod(c, T_SHARDS)
        out[t * MS:(t + 1) * MS, f * DS:(f + 1) * DS] = \
            results[c]["outT"].T.astype(np.float32)
    return out.reshape(B, N, D_MODEL)


def run(boundary_tokens, W_b2b, b_b2b, mode=None, **spmd_kwargs):
    mode = mode or MODE
    nc = _get_nc(mode)
    in_maps = _make_in_maps(boundary_tokens, W_b2b, b_b2b)
    res = run_bass_kernel_spmd(nc, in_maps, list(range(NCORES)), **spmd_kwargs)
    return _assemble(res.results), res


def kernel(boundary_tokens, W_b2b, b_b2b):
    out, _ = run(boundary_tokens, W_b2b, b_b2b)
    return out
